# revision 11
# baseline (speedup 1.0000x reference)
"""Trainium2 Bass kernel for 3x3 valid conv (C_in=8, C_out=8, H=W=2048).

Strategy (2-D spatial sharding, 4 H-bands x 2 W-halves across 8 cores):
  - Host splits the image into a 4x2 grid: per core ~512 output rows
    (+2 halo) x 1023 output cols (+2 halo), and packs each slab into the
    SBUF layout the TensorE wants:
        xp[(ci, r), b, w] = slab[ci, h0(b) + r, w]
    for 37 row-blocks b of J=14 output rows (h0 = 14*b, last block
    overlap-recomputes), r = 0..15. Halo rows are duplicated host-side so
    every device load is a plain contiguous slice.
  - On-wire dtypes: activations go over HBM as float8e3 (e3m4, 1 B/elem;
    measured end-to-end rel err 1.45e-2 vs the 2e-2 budget), weights stay
    fp16 and the TensorE runs mixed operand dtypes (fp16 lhsT x fp8e3 rhs
    -> fp32 PSUM, verified bit-accurate on HW). Outputs remain fp16.
  - Three lhsT weight matrices (one per kw) of shape [K=128, M=112]:
        K = (ci, r), M = (co, j), lhsT[kw][ci*16+r, co*14+j] =
        W[co, ci, r-j, kw] for 0 <= r-j <= 2.
  - Device per core: per block, one DMA loads [128, 1025] fp8 (first block
    split in two so the PE starts earlier); per column tile (512 + 511),
    3 accumulating matmuls (kw = 0,1,2; rhs shifted along the free axis)
    produce [112, <=512] fp32 PSUM; PSUM tiles are copied (DVE/ACT,
    fp32->fp16) and stored per block. Host scatters op back to
    (C, 2046, 2046) fp32.

  The schedule is PE-bound: the 4x2 grid needs 37*2*3 = 222
  ldweights+matmul pairs/core (vs 228 for 8x1) at ~244 ns each
  (~512 cols * 0.417 ns + ~30 ns fixed per-pair overhead) ~= 54 us, with
  DMA (~13 MB/core at ~360 GB/s ~= 36 us) fully overlapped. TimelineSim:
  56045 ns; measured in-situ HW matmul-loop rate agrees within a few %.
"""
import numpy as np
import ml_dtypes

import concourse.mybir as mybir
import concourse.tile as tile
from concourse import bacc
from concourse.bass_utils import run_bass_kernel_spmd

C = 8
H = W = 2048
KH = KW = 3
H_OUT = W_OUT = 2046
N_CORES = 8

HB = 4                 # H bands
WB = 2                 # W halves
BAND_ROWS = [512, 512, 512, 510]    # output rows per band (sum 2046)
COL_W = 1023           # output cols per W-half
WIN = COL_W + 2        # 1025 input cols per core

J = 14
R = 16
K = C * R              # 128
M = C * J              # 112
NBLK = 37              # ceil(512/14)
COL_TILES = [(0, 512), (512, 511)]

IN_DT = mybir.dt.float8e3
IN_NP = ml_dtypes.float8_e3m4
W_DT = mybir.dt.float16
W_NP = np.float16
OUT_DT = mybir.dt.float16

Y_BUFS = 8
O_BUFS = 4
FIRST_CHUNKS = 2


def block_starts(rows):
    return [J * b for b in range(NBLK - 1)] + [rows - J]


def build_nc(repeat: int = 1, mode: str = "full"):
    do_mm = mode in ("full", "nocopy")
    do_copy = mode in ("full",)
    do_dma = mode in ("full", "nocopy", "dma")
    nc = bacc.Bacc("TRN2", target_bir_lowering=False, debug=False,
                   num_devices=N_CORES)
    xp = nc.dram_tensor("xp", [K, NBLK, WIN], IN_DT, kind="ExternalInput").ap()
    wts = nc.dram_tensor("wts", [KW, K, M], W_DT, kind="ExternalInput").ap()
    op = nc.dram_tensor("op", [M, NBLK, COL_W], OUT_DT, kind="ExternalOutput").ap()

    with tile.TileContext(nc) as tc:
        with (
            tc.tile_pool(name="wpool", bufs=1) as wpool,
            tc.tile_pool(name="ypool", bufs=Y_BUFS) as ypool,
            tc.tile_pool(name="opool", bufs=O_BUFS) as opool,
            tc.tile_pool(name="pspool", bufs=8, space="PSUM") as pspool,
        ):
            wsb = wpool.tile([K, KW * M], W_DT)
            for kw in range(KW):
                nc.sync.dma_start(wsb[:, kw * M:(kw + 1) * M], wts[kw])

            for rep_i in range(repeat):
                for b in range(NBLK):
                    yt = ypool.tile([K, WIN], IN_DT, name="y", tag="y")
                    if do_dma:
                        if rep_i == 0 and b == 0 and FIRST_CHUNKS > 1:
                            # split at 516 so the tile-0 chains (cols 0..514
                            # for kw<=2) depend only on the first chunk
                            bounds = [0, 516, WIN]
                            for c in range(FIRST_CHUNKS):
                                nc.gpsimd.dma_start(
                                    yt[:, bounds[c]:bounds[c + 1]],
                                    xp[:, 0, bounds[c]:bounds[c + 1]])
                        else:
                            nc.gpsimd.dma_start(yt[:], xp[:, b:b + 1, :])

                    o = opool.tile([M, COL_W], OUT_DT, name="o", tag="o")
                    pss = [pspool.tile([M, 512], mybir.dt.float32,
                                       name=f"ps{ti}", tag="ps")
                           for ti in range(len(COL_TILES))]
                    if do_mm:
                        for kw in range(KW):
                            for ti, (w0, n) in enumerate(COL_TILES):
                                nc.tensor.matmul(
                                    pss[ti][:, :n],
                                    lhsT=wsb[:, kw * M:(kw + 1) * M],
                                    rhs=yt[:, w0 + kw:w0 + kw + n],
                                    start=(kw == 0),
                                    stop=(kw == KW - 1),
                                )
                    if do_copy:
                        for ti, (w0, n) in enumerate(COL_TILES):
                            dst = o[:, w0:w0 + n]
                            if ti % 2 == 0:
                                nc.vector.tensor_copy(dst, pss[ti][:, :n])
                            else:
                                nc.scalar.copy(dst, pss[ti][:, :n])
                    if not do_copy and do_dma:
                        nc.vector.memset(o[:, :8], 0.0)
                    if do_dma:
                        nc.sync.dma_start(op[:, b, :], o[:])

    nc.compile()
    return nc


def build_weight_lhst(weight: np.ndarray) -> np.ndarray:
    wl = np.zeros((KW, K, M), np.float32)
    for kw in range(KW):
        for co in range(C):
            for j in range(J):
                for kh in range(KH):
                    r = j + kh
                    wl[kw, np.arange(C) * R + r, co * J + j] = weight[co, :, kh, kw]
    return wl.astype(W_NP)


def pack_core_input(slab: np.ndarray, rows: int) -> np.ndarray:
    """slab: (C, rows+2, WIN) e3m4 -> xp (K, NBLK, WIN)."""
    s0, s1, s2 = slab.strides
    v = np.lib.stride_tricks.as_strided(
        slab, shape=(C, R, NBLK - 1, WIN), strides=(s0, s1, J * s1, s2))
    xp = np.empty((C, R, NBLK, WIN), slab.dtype)
    xp[:, :, :NBLK - 1, :] = v
    ls = rows - J
    xp[:, :, NBLK - 1, :] = slab[:, ls:ls + R, :]
    return xp.reshape(K, NBLK, WIN)


def unpack_core_output(op: np.ndarray, rows: int) -> np.ndarray:
    op = op.reshape(C, J, NBLK, COL_W)
    res = np.empty((C, rows, COL_W), np.float32)
    res[:, :J * (NBLK - 1), :] = (
        op[:, :, :NBLK - 1, :].transpose(0, 2, 1, 3).reshape(C, J * (NBLK - 1), COL_W))
    res[:, rows - J:, :] = op[:, :, NBLK - 1, :].astype(np.float32)
    return res


def shard_inputs(x: np.ndarray, weight: np.ndarray):
    xc = np.ascontiguousarray(x).astype(IN_NP)
    wl = build_weight_lhst(weight)
    in_maps = []
    for cid in range(N_CORES):
        hb, wh = cid // WB, cid % WB
        rows = BAND_ROWS[hb]
        rlo = sum(BAND_ROWS[:hb])
        clo = wh * COL_W
        slab = xc[:, rlo:rlo + rows + 2, clo:clo + WIN]
        in_maps.append({"xp": pack_core_input(slab, rows), "wts": wl})
    return in_maps


def unshard_output(results) -> np.ndarray:
    out = np.empty((C, H_OUT, W_OUT), np.float32)
    for cid in range(N_CORES):
        hb, wh = cid // WB, cid % WB
        rows = BAND_ROWS[hb]
        rlo = sum(BAND_ROWS[:hb])
        clo = wh * COL_W
        out[:, rlo:rlo + rows, clo:clo + COL_W] = \
            unpack_core_output(results[cid]["op"], rows)
    return out


_NC_CACHE = None


def _get_nc():
    global _NC_CACHE
    if _NC_CACHE is None:
        _NC_CACHE = build_nc()
    return _NC_CACHE


def run(inputs: dict, **spmd_kwargs):
    in_maps = shard_inputs(np.asarray(inputs["x"]), np.asarray(inputs["weight"]))
    nc = _get_nc()
    res = run_bass_kernel_spmd(nc, in_maps, core_ids=list(range(N_CORES)), **spmd_kwargs)
    return unshard_output(res.results).astype(np.float32), res


def kernel(**inputs) -> np.ndarray:
    out, _ = run(inputs)
    return out
